# revision 36
# baseline (speedup 1.0000x reference)
"""DFine multihead attention on 8 Trainium2 NeuronCores (Bass/Tile).

Problem: B=4, S=2048, D=256, H=8, HD=32.
    hp = hidden + pos
    q = hp @ Wq, k = hp @ Wk (per head), v = hidden @ Wv
    scores = (q*HD^-0.5) @ k^T + mask ; attn = softmax(scores)
    out = (attn @ v reshaped) @ Wo + bo

Sharding: core c handles (b = c % 4, head-group hg = c // 4) -> 4 heads each.
Each core returns a partial out (its heads' slice of the D contraction of Wo);
host sums the two head-group partials per batch.

Kernel structure per core (all on-device):
  1. Load hidden/pos s-tiles, PE-transpose into hiddenT/hpT ([d, s] layouts).
     Weights are fp32r-rounded once via DVE copies (walrus requires rounded
     producers for fp32r matmul operands).
  2. Projections (fp32r matmuls, 4x the fp32 PE rate at N>=256): qT/kT
     stacks [4h*32, 2048] via full-width stationaries; v_stack in natural
     [k, he] bf16 layout with a constant ones column per head.
  3. Main loop over (q-block n of 512, k-tile m of 128, head-pair):
     scores^T = k_m q_n^T via row-tiled fp32r K=32 matmuls -> PSUM
     (fp32r forbids col tiling, row tiling is fine),
     exp via ScalarE (fused *HD^-0.5) -> bf16 SBUF,
     ctx^T (+denom) += [v | 1]^T exp: bf16 33-col stationaries col-tiled at
     (0,0)/(0,64), so PSUM rows 32/96 accumulate softmax denominators for
     free (no separate denominator matmuls).
  4. Per q-block: reciprocal of the 4 denom rows (DVE), stream_shuffle
     broadcast, per-band normalize (DVE), fp32r out-projection against the
     Wo slice, bias add, DMA out.

softmax is computed without max-subtraction: scores here are ~N(0, 4) so
exp() stays well within fp32 range; identical result up to fp rounding.
bf16 on the attn*v path costs ~3e-3 relative error (gate is 2e-2).
"""

from contextlib import ExitStack

import numpy as np

import concourse.bass as bass
import concourse.mybir as mybir
import concourse.tile as tile
from concourse import bacc, bass_utils
from concourse.bass import ds, ts
from concourse.masks import make_identity
from concourse import library_config

B, S, D, H = 4, 2048, 256, 8
HD = D // H            # 32
HPG = 4                # heads per group (per core)
HG = H // HPG          # 2 head groups
SCALING = HD ** -0.5
NT = S // 128          # 16 s-tiles
NB = S // 512          # 4 q-blocks
DT = D // 128          # 2 d-tiles
F32 = mybir.dt.float32
F32R = mybir.dt.float32r   # full-rate PE dtype (1 cyc/row at N>=256 vs 4 for fp32)
BF16 = mybir.dt.bfloat16   # used on the attn*v path (fp32r forbids col tiling)
N_CORES = 8

_cached = {}


def _r(ap):
    # bitcast an fp32 AP to float32r for matmul operands
    return ap.bitcast(F32R)


def _build_nc(reps=1):
    nc = bacc.Bacc("TRN2", target_bir_lowering=False, debug=False,
                   num_devices=N_CORES)

    hidden = nc.declare_dram_parameter("hidden", [S, D], F32, isOutput=False).ap()
    pos = nc.declare_dram_parameter("pos", [S, D], F32, isOutput=False).ap()
    wq = nc.declare_dram_parameter("wq", [D, HPG * HD], F32, isOutput=False).ap()
    wk = nc.declare_dram_parameter("wk", [D, HPG * HD], F32, isOutput=False).ap()
    wv = nc.declare_dram_parameter("wv", [D, HPG * HD], F32, isOutput=False).ap()
    bq = nc.declare_dram_parameter("bq", [HPG * HD], F32, isOutput=False).ap()
    bk = nc.declare_dram_parameter("bk", [HPG * HD], F32, isOutput=False).ap()
    bv = nc.declare_dram_parameter("bv", [HPG * HD], F32, isOutput=False).ap()
    wo = nc.declare_dram_parameter("wo", [HPG * HD, D], F32, isOutput=False).ap()
    bo = nc.declare_dram_parameter("bo", [D], F32, isOutput=False).ap()
    out = nc.declare_dram_parameter("out", [S, D], F32, isOutput=True).ap()

    def bcast_dram(ap, n_part):
        # DMA-broadcast a 1D DRAM vector across n_part partitions.
        return bass.AP(tensor=ap.tensor, offset=ap.offset,
                       ap=[[0, n_part]] + list(ap.ap))

    with tile.TileContext(nc) as tc, ExitStack() as stack:
        # ---- persistent SBUF ----
        pers = stack.enter_context(tc.tile_pool(name="persist", bufs=1))
        wq_sb = pers.tile([128, DT, HPG * HD], F32, name="wq_sb")
        wk_sb = pers.tile([128, DT, HPG * HD], F32, name="wk_sb")
        wv_sb = pers.tile([128, DT, HPG * HD], F32, name="wv_sb")
        wo_sb = pers.tile([128, D], F32, name="wo_sb")
        bq_sb = pers.tile([128, 1], F32, name="bq_sb")
        bk_sb = pers.tile([128, 1], F32, name="bk_sb")
        bv_bc = pers.tile([128, HPG * HD], F32, name="bv_bc")
        bo_bc = pers.tile([128, D], F32, name="bo_bc")
        ident = pers.tile([128, 128], F32, name="ident")
        hiddenT = pers.tile([128, DT, S], F32, name="hiddenT")
        hpT = pers.tile([128, DT, S], F32, name="hpT")
        qT = pers.tile([128, S], F32, name="qT")
        kT = pers.tile([128, S], F32, name="kT")
        # v in natural [k, (h, 32 v + 1 ones)] layout, per k-tile (bf16: the
        # ones column makes the ctx matmul also produce softmax denominators)
        vstack = pers.tile([128, NT, HPG * (HD + 1)], BF16, name="vstack")

        wq_rr = pers.tile([128, DT, HPG * HD], F32, name="wq_rr")
        wk_rr = pers.tile([128, DT, HPG * HD], F32, name="wk_rr")
        wv_rr = pers.tile([128, DT, HPG * HD], F32, name="wv_rr")
        wo_rr = pers.tile([128, D], F32, name="wo_rr")
        for dt in range(DT):
            nc.sync.dma_start(out=wq_sb[:, dt, :], in_=wq[ts(dt, 128), :])
            nc.sync.dma_start(out=wk_sb[:, dt, :], in_=wk[ts(dt, 128), :])
            nc.sync.dma_start(out=wv_sb[:, dt, :], in_=wv[ts(dt, 128), :])
        nc.sync.dma_start(out=wo_sb, in_=wo)
        # fp32r-round the DMA'd weights (walrus requires rounded producers
        # for fp32r matmul operands)
        nc.vector.tensor_copy(_r(wq_rr), wq_sb)
        nc.vector.tensor_copy(_r(wk_rr), wk_sb)
        nc.vector.tensor_copy(_r(wv_rr), wv_sb)
        nc.vector.tensor_copy(_r(wo_rr), wo_sb)
        nc.sync.dma_start(out=bq_sb, in_=bq.rearrange("(p one) -> p one", one=1))
        nc.sync.dma_start(out=bk_sb, in_=bk.rearrange("(p one) -> p one", one=1))
        nc.gpsimd.dma_start(out=bv_bc, in_=bcast_dram(bv, 128))
        nc.gpsimd.dma_start(out=bo_bc, in_=bcast_dram(bo, 128))
        make_identity(nc, ident)

        vstack_r = vstack.rearrange("p m (h c) -> p m h c", c=HD + 1)
        nc.vector.memset(vstack_r[:, :, :, HD:HD + 1], 1.0)

        # ---- prep: transposes + projections ----
        with tc.tile_pool(name="prep_psum", bufs=1, space="PSUM") as pp, \
             tc.tile_pool(name="io", bufs=4) as io:
            # transposes: hiddenT, hpT  (hpT = hiddenT + posT)
            for g in range(NT // 4):
                ps_trh = [pp.tile([128, 512], F32, name=f"ps_trh{dt}", bufs=1)
                          for dt in range(DT)]
                ps_trp = [pp.tile([128, 512], F32, name=f"ps_trp{dt}", bufs=1)
                          for dt in range(DT)]
                for j in range(4):
                    m = 4 * g + j
                    hid_t = io.tile([128, D], F32, name="hid_t")
                    nc.sync.dma_start(out=hid_t, in_=hidden[ts(m, 128), :])
                    pos_t = io.tile([128, D], F32, name="pos_t")
                    nc.sync.dma_start(out=pos_t, in_=pos[ts(m, 128), :])
                    for dt in range(DT):
                        nc.tensor.transpose(ps_trh[dt][:, ts(j, 128)],
                                            hid_t[:, ts(dt, 128)], ident)
                        nc.tensor.transpose(ps_trp[dt][:, ts(j, 128)],
                                            pos_t[:, ts(dt, 128)], ident)
                for dt in range(DT):
                    nc.vector.tensor_copy(_r(hiddenT[:, dt, ts(g, 512)]),
                                          ps_trh[dt])
                    nc.vector.tensor_add(_r(hpT[:, dt, ts(g, 512)]), ps_trp[dt],
                                         hiddenT[:, dt, ts(g, 512)])

            # v_stack
            for m in range(NT):
                ps_v = pp.tile([128, HPG * HD], F32, name="ps_v", bufs=2)
                for dt in range(DT):
                    nc.tensor.matmul(ps_v, lhsT=_r(hiddenT[:, dt, ts(m, 128)]),
                                     rhs=_r(wv_rr[:, dt, :]),
                                     start=(dt == 0), stop=(dt == DT - 1))
                nc.vector.tensor_add(
                    vstack_r[:, m, :, 0:HD],
                    ps_v.rearrange("p (h c) -> p h c", c=HD),
                    bv_bc.rearrange("p (h c) -> p h c", c=HD))

            # qT / kT stacks: one full-width matmul covers all 4 heads
            # (output partition he = 32h+e matches the packed band layout)
            for n in range(NB):
                for (w_sb, b_sb, dest) in ((wq_rr, bq_sb, qT), (wk_rr, bk_sb, kT)):
                    ps_qk = pp.tile([128, 512], F32, name="ps_qk", bufs=2)
                    for dt in range(DT):
                        nc.tensor.matmul(
                            ps_qk, lhsT=_r(w_sb[:, dt, :]),
                            rhs=_r(hpT[:, dt, ts(n, 512)]),
                            start=(dt == 0), stop=(dt == DT - 1))
                    nc.vector.tensor_scalar(
                        out=_r(dest[:, ts(n, 512)]), in0=ps_qk,
                        scalar1=b_sb, scalar2=None, op0=mybir.AluOpType.add)

        # ---- main attention loop ----
        with tc.tile_pool(name="sc_psum", bufs=2, space="PSUM") as scp, \
             tc.tile_pool(name="acc_psum", bufs=1, space="PSUM") as accp, \
             tc.tile_pool(name="sb_main", bufs=6) as sbm, \
             tc.tile_pool(name="sb_small", bufs=2) as sbs:
          def _main_body(_iv=None):
            for n in range(NB):
                ps_ctxAB = [accp.tile([128, 512], F32, name=f"ps_ctx{i}")
                            for i in range(2)]

                def _ctx_den(m, half, expt):
                    # ctx^T (+ denom row 32) accumulation for one
                    # (m, head-pair); the 33-col stationary includes the ones
                    # column, so row 32/96 of PSUM accumulates the softmax
                    # denominator. Emitted a few half-steps behind the
                    # producing exp so the PE never stalls on ScalarE.
                    ps_ctx = ps_ctxAB[half]
                    for j in range(2):
                        h = 2 * half + j
                        nc.tensor.matmul(
                            ps_ctx[ds(64 * j, HD + 1), :],
                            lhsT=vstack_r[:, m, h, 0:HD + 1],
                            rhs=expt[:, ds(512 * j, 512)],
                            start=(m == 0), stop=(m == NT - 1),
                            tile_position=(0, 64 * j),
                            skip_group_check=True)

                pend = []
                for m in range(NT):
                    for half in range(2):
                        ps_sc = scp.tile([128, 1024], F32, name="ps_sc")
                        for j in range(2):
                            h = 2 * half + j
                            nc.tensor.matmul(
                                ps_sc[:, ds(512 * j, 512)],
                                lhsT=_r(kT[ds(32 * h, 32), ts(m, 128)]),
                                rhs=_r(qT[ds(32 * h, 32), ts(n, 512)]),
                                start=True, stop=True,
                                tile_position=(32 * h, 0))
                        expt = sbm.tile([128, 1024], BF16, name="expt")
                        nc.scalar.activation(expt, ps_sc,
                                             mybir.ActivationFunctionType.Exp,
                                             scale=SCALING)
                        pend.append((m, half, expt))
                        if len(pend) > 3:
                            _ctx_den(*pend.pop(0))
                for p in pend:
                    _ctx_den(*p)

                # normalize + out-projection for this q-block; denominators
                # sit at partition 32/96 of each ctx PSUM tile
                recip = sbs.tile([128, 512], F32, name="recip")
                nc.vector.memset(recip, 1.0)
                for h in range(HPG):
                    nc.vector.reciprocal(
                        recip[ds(32 * h, 1), :],
                        ps_ctxAB[h // 2][ds(64 * (h % 2) + HD, 1), :])
                rbc = sbs.tile([128, 512], F32, name="rbc")
                # broadcast row 0 of each 32-partition group (the per-head
                # 1/denom rows) to the whole group
                nc.vector.stream_shuffle(rbc, recip, [0] * 32)
                ctxn = sbs.tile([128, 512], F32, name="ctxn")
                for h in range(HPG):
                    nc.vector.tensor_mul(
                        _r(ctxn[ds(32 * h, 32), :]),
                        ps_ctxAB[h // 2][ds(64 * (h % 2), 32), :],
                        rbc[ds(32 * h, 32), :])
                for st in range(4):
                    ps_out = accp.tile([128, D], F32, name="ps_out", bufs=2)
                    nc.tensor.matmul(ps_out, lhsT=_r(ctxn[:, ts(st, 128)]),
                                     rhs=_r(wo_rr), start=True, stop=True)
                    osb = sbm.tile([128, D], F32, name="osb")
                    nc.vector.tensor_add(osb, ps_out, bo_bc)
                    nc.sync.dma_start(out=out[ds(512 * n + 128 * st, 128), :],
                                      in_=osb)
          if reps == 1:
              _main_body()
          else:
              with tc.For_i(0, reps, 1) as iv:
                  _main_body(iv)
    nc.compile()
    return nc


def _get_nc(reps=1):
    key = f"nc{reps}"
    if key not in _cached:
        _cached[key] = _build_nc(reps)
    return _cached[key]


def _reference_numpy(hidden_states, position_embeddings, attention_mask,
                     Wq, bq, Wk, bk, Wv, bv, Wo, bo):
    # Fallback for nonzero attention_mask (never hit for this problem's spec).
    hp = hidden_states + position_embeddings
    q = np.einsum("bsd,dhe->bshe", hp, Wq) + bq
    k = np.einsum("bsd,dhe->bshe", hp, Wk) + bk
    v = np.einsum("bsd,dhe->bshe", hidden_states, Wv) + bv
    q = q * SCALING
    scores = np.einsum("bqhe,bkhe->bhqk", q, k) + attention_mask[:, None]
    scores -= scores.max(axis=-1, keepdims=True)
    e = np.exp(scores)
    attn = e / e.sum(axis=-1, keepdims=True)
    ctx = np.einsum("bhqk,bkhe->bqhe", attn, v).reshape(B, S, D)
    return (np.einsum("bsd,de->bse", ctx, Wo) + bo).astype(np.float32)


def expected_core0(np_inputs):
    # partial output computed by core 0 (b=0, heads 0-3, includes bo)
    hs = np_inputs["hidden_states"][0]
    hp = hs + np_inputs["position_embeddings"][0]
    q = np.einsum("sd,dhe->she", hp, np_inputs["Wq"][:, :HPG]) * SCALING
    k = np.einsum("sd,dhe->she", hp, np_inputs["Wk"][:, :HPG])
    v = np.einsum("sd,dhe->she", hs, np_inputs["Wv"][:, :HPG])
    sc = np.einsum("qhe,khe->hqk", q, k)
    e = np.exp(sc - sc.max(axis=-1, keepdims=True))
    attn = e / e.sum(axis=-1, keepdims=True)
    ctx = np.einsum("hqk,khe->qhe", attn, v).reshape(S, HPG * HD)
    return (ctx @ np_inputs["Wo"][:HPG * HD] + np_inputs["bo"]).astype(np.float32)


def make_in_maps(np_inputs):
    hidden_states = np_inputs["hidden_states"]
    position_embeddings = np_inputs["position_embeddings"]
    Wq, bq = np_inputs["Wq"], np_inputs["bq"]
    Wk, bk = np_inputs["Wk"], np_inputs["bk"]
    Wv, bv = np_inputs["Wv"], np_inputs["bv"]
    Wo, bo = np_inputs["Wo"], np_inputs["bo"]
    in_maps = []
    for c in range(N_CORES):
        b, hg = c % B, c // B
        hs = slice(hg * HPG, (hg + 1) * HPG)
        in_maps.append({
            "hidden": np.ascontiguousarray(hidden_states[b]),
            "pos": np.ascontiguousarray(position_embeddings[b]),
            "wq": np.ascontiguousarray(Wq[:, hs, :]).reshape(D, HPG * HD),
            "wk": np.ascontiguousarray(Wk[:, hs, :]).reshape(D, HPG * HD),
            "wv": np.ascontiguousarray(Wv[:, hs, :]).reshape(D, HPG * HD),
            "bq": np.ascontiguousarray(bq[hs]).reshape(HPG * HD),
            "bk": np.ascontiguousarray(bk[hs]).reshape(HPG * HD),
            "bv": np.ascontiguousarray(bv[hs]).reshape(HPG * HD),
            "wo": np.ascontiguousarray(Wo[hg * HPG * HD:(hg + 1) * HPG * HD, :]),
            "bo": bo if hg == 0 else np.zeros_like(bo),
        })
    return in_maps


def kernel(hidden_states, position_embeddings, attention_mask,
           Wq, bq, Wk, bk, Wv, bv, Wo, bo, _want_results=False,
           _trace=False, _tmpdir=None):
    args = [np.asarray(a, dtype=np.float32) for a in
            (hidden_states, position_embeddings, attention_mask,
             Wq, bq, Wk, bk, Wv, bv, Wo, bo)]
    (hidden_states, position_embeddings, attention_mask,
     Wq, bq, Wk, bk, Wv, bv, Wo, bo) = args

    if np.any(attention_mask):
        return _reference_numpy(hidden_states, position_embeddings,
                                attention_mask, Wq, bq, Wk, bk, Wv, bv, Wo, bo)

    nc = _get_nc()
    in_maps = make_in_maps({
        "hidden_states": hidden_states, "position_embeddings": position_embeddings,
        "Wq": Wq, "bq": bq, "Wk": Wk, "bk": bk, "Wv": Wv, "bv": bv,
        "Wo": Wo, "bo": bo})
    res = bass_utils.run_bass_kernel_spmd(nc, in_maps, list(range(N_CORES)),
                                          trace=_trace, tmpdir=_tmpdir)
    out = np.empty((B, S, D), np.float32)
    for b in range(B):
        out[b] = res.results[b]["out"] + res.results[b + B]["out"]
    if _want_results:
        return out, res
    return out

